# revision 12
# baseline (speedup 1.0000x reference)
"""Banded (lookahead) cross-attention on 8 Trainium2 NeuronCores.

Reference computation (B=4, T=2048, D=1024, H=16, hd=64):
    Q = query @ Wq.T + bq ; K = key_value @ Wk.T + bk ; V = key_value @ Wv.T + bv
    scores = Q K^T / sqrt(hd), masked to j <= i + lookahead
    out = softmax(scores) V, concat heads, @ Wo.T + bo
Sharding: 8 cores = (batch b = c//2) x (head-half = c%2, 8 heads each).
Host sums the two outT partials per batch and adds bo.

v2 design (single fused pipeline, all engines busy):
  - Attention phase D is scalar-bound (exp on ACT @1.2GHz). All other
    scalar work (bias-adds, psum->sbuf copies) moved to DVE so ACT does
    exp only.
  - QK^T packs both heads of a pair concurrently on the PE: per-head
    contraction is hd=64, and bass auto-derives tile_position from the
    base partition (head A rows 0:64 -> row tiles 0-1, head B rows
    64:128 -> tiles 2-3), so back-to-back issue runs them in parallel.
  - Projection/output matmul bundles (A/B/C/E) are interleaved into D's
    emission as tensor-engine filler, keeping the PE busy (HAM warm)
    while ACT chews exp.
  - Everything bf16 (x, weights, P, V, aT, Wo); psum f32; div chain
    recip(DVE) -> partition_broadcast(GPSIMD) -> mult(DVE).
"""

import sys

for _p in ("/opt/trn_rl_repo", "/opt/pypackages"):
    if _p not in sys.path:
        sys.path.append(_p)

import numpy as np
import ml_dtypes

import concourse.bass as bass
import concourse.tile as tile
from concourse import bacc, mybir
from concourse.bass_utils import run_bass_kernel_spmd

F32 = mybir.dt.float32
BF16 = mybir.dt.bfloat16
AF = mybir.ActivationFunctionType
MUL = mybir.AluOpType.mult

B, T, D = 4, 2048, 1024
H, HD = 16, 64
H_LOC = 8                    # heads per core
E_LOC = H_LOC * HD           # 512 projected dims per core
NJB = T // 128               # 16 j-blocks
NIC = T // 512               # 4 i-chunks
NDT = D // 128               # 8 contraction tiles
NET = E_LOC // 128           # 4 e-tiles (head-pairs)
SCALE = HD ** -0.5
VW = H_LOC * (HD + 1)        # 520 v_sb layout width
VH = HD + 1                  # 65

_CACHE = {}


def _groups(L):
    """Per i-chunk: list of (jb, delta, masked); delta = first valid column
    offset inside the 512-wide chunk (0 for dense)."""
    out = []
    deltas = set()
    for ic in range(NIC):
        i0 = 512 * ic
        lst = []
        for jb in range(NJB):
            j0 = 128 * jb
            if i0 + 511 + L < j0:
                break                          # fully masked from here on
            if j0 + 127 <= i0 + L:
                lst.append((jb, 0, False))     # dense
            else:
                d = j0 - L - i0
                lst.append((jb, max(d, 0), True))
                deltas.add(d)
        out.append(lst)
    return out, sorted(deltas)


def _build(L):
    groups, deltas = _groups(L)
    dpos = {d: k for k, d in enumerate(deltas)}
    nmask = max(1, len(deltas))

    nc = bacc.Bacc("TRN2", target_bir_lowering=False, debug=False)
    xqT = nc.dram_tensor("xqT", [D, T], BF16, kind="ExternalInput").ap()
    xkvT = nc.dram_tensor("xkvT", [D, T], BF16, kind="ExternalInput").ap()
    wqT = nc.dram_tensor("wqT", [D, E_LOC], BF16, kind="ExternalInput").ap()
    wkT = nc.dram_tensor("wkT", [D, E_LOC], BF16, kind="ExternalInput").ap()
    wvT = nc.dram_tensor("wvT", [D, E_LOC], BF16, kind="ExternalInput").ap()
    woT = nc.dram_tensor("woT", [E_LOC, D], BF16, kind="ExternalInput").ap()
    bq4 = nc.dram_tensor("bq4", [128, NET], F32, kind="ExternalInput").ap()
    bk4 = nc.dram_tensor("bk4", [128, NET], F32, kind="ExternalInput").ap()
    bv_row = nc.dram_tensor("bv_row", [1, E_LOC], BF16, kind="ExternalInput").ap()
    masks = nc.dram_tensor("masks", [128, nmask * 512], BF16,
                           kind="ExternalInput").ap()
    outT = nc.dram_tensor("outT", [D, T], F32, kind="ExternalOutput").ap()

    with tile.TileContext(nc) as tc:
        with tc.tile_pool(name="small", bufs=1) as small, \
             tc.tile_pool(name="persist", bufs=1) as persist, \
             tc.tile_pool(name="slabs", bufs=1) as slabs, \
             tc.tile_pool(name="ptp", bufs=7) as pt_pool, \
             tc.tile_pool(name="dv", bufs=2) as dv_pool, \
             tc.tile_pool(name="os", bufs=4) as os_pool, \
             tc.tile_pool(name="pps", bufs=2, space="PSUM") as pps, \
             tc.tile_pool(name="sps", bufs=2, space="PSUM") as sps, \
             tc.tile_pool(name="ops", bufs=2, space="PSUM") as ops:

            # ---- DMA issue order == first-need order ----
            wq_sb = [slabs.tile([128, E_LOC], BF16, tag=f"wq{d}", name=f"wq{d}")
                     for d in range(NDT)]
            for d in range(NDT):
                nc.sync.dma_start(wq_sb[d][:], wqT[128 * d:128 * (d + 1), :])
            xq_sb = {}
            xkv_sb = {}
            for t in range(NIC):
                for d in range(NDT):
                    xq_sb[(d, t)] = slabs.tile(
                        [128, 512], BF16, tag=f"xq{d}_{t}", name=f"xq{d}_{t}")
                    xkv_sb[(d, t)] = slabs.tile(
                        [128, 512], BF16, tag=f"xkv{d}_{t}", name=f"xkv{d}_{t}")
            wk_sb = [slabs.tile([128, E_LOC], BF16, tag=f"wk{d}", name=f"wk{d}")
                     for d in range(NDT)]
            wv_sb = [slabs.tile([128, E_LOC], BF16, tag=f"wv{d}", name=f"wv{d}")
                     for d in range(NDT)]
            wo_sb = [slabs.tile([128, D], BF16, tag=f"wo{e}", name=f"wo{e}")
                     for e in range(NET)]
            for d in range(NDT):
                nc.sync.dma_start(
                    xq_sb[(d, 0)][:], xqT[128 * d:128 * (d + 1), 0:512])
            for d in range(NDT):
                nc.sync.dma_start(wk_sb[d][:], wkT[128 * d:128 * (d + 1), :])
            for d in range(NDT):
                nc.sync.dma_start(
                    xkv_sb[(d, 0)][:], xkvT[128 * d:128 * (d + 1), 0:512])
            for d in range(NDT):
                nc.sync.dma_start(wv_sb[d][:], wvT[128 * d:128 * (d + 1), :])
            bq_sb = small.tile([128, NET], F32, tag="bq")
            bk_sb = small.tile([128, NET], F32, tag="bk")
            bv_sb = small.tile([1, E_LOC], BF16, tag="bv")
            on_sb = small.tile([1, 128], BF16, tag="on")
            mk_sb = persist.tile([128, nmask * 512], BF16, tag="mk")
            nc.sync.dma_start(mk_sb[:], masks[:])
            nc.sync.dma_start(bq_sb[:], bq4[:])
            nc.sync.dma_start(bk_sb[:], bk4[:])
            nc.sync.dma_start(bv_sb[:], bv_row[:])
            for t in range(1, NIC):
                for d in range(NDT):
                    nc.sync.dma_start(
                        xq_sb[(d, t)][:],
                        xqT[128 * d:128 * (d + 1), 512 * t:512 * (t + 1)])
                for d in range(NDT):
                    nc.sync.dma_start(
                        xkv_sb[(d, t)][:],
                        xkvT[128 * d:128 * (d + 1), 512 * t:512 * (t + 1)])
            for e in range(NET):
                nc.sync.dma_start(wo_sb[e][:], woT[128 * e:128 * (e + 1), :])

            qT = [persist.tile([128, T], BF16, tag=f"qt{i}", name=f"qt{i}")
                  for i in range(NET)]
            kT = [persist.tile([128, T], BF16, tag=f"kt{i}", name=f"kt{i}")
                  for i in range(NET)]
            v_sb = [persist.tile([128, VW], BF16, tag=f"v{i}", name=f"v{i}")
                    for i in range(NJB)]
            aT = [persist.tile([128, T], BF16, tag=f"at{i}", name=f"at{i}")
                  for i in range(NET)]

            # ones: lhsT row for C's bias matmul; v ones columns (softmax
            # denominator accumulators) written once
            nc.gpsimd.memset(on_sb[:], 1.0)
            for tt in range(NJB):
                vv = v_sb[tt][:].rearrange("p (h w) -> p h w", w=VH)
                nc.gpsimd.memset(vv[:, :, HD:VH], 1.0)

            # ---- filler bundle emitters (projections / output) ----
            def emit_A(et, t):
                ps = pps.tile([128, 512], F32, tag="pp")
                for d in range(NDT):
                    nc.tensor.matmul(
                        ps[:], wq_sb[d][:, 128 * et:128 * (et + 1)],
                        xq_sb[(d, t)][:], start=(d == 0), stop=(d == NDT - 1))
                nc.vector.tensor_scalar_add(
                    qT[et][:, 512 * t:512 * (t + 1)], ps[:],
                    bq_sb[:, et:et + 1])

            def emit_B(et, t):
                ps = pps.tile([128, 512], F32, tag="pp")
                for d in range(NDT):
                    nc.tensor.matmul(
                        ps[:], wk_sb[d][:, 128 * et:128 * (et + 1)],
                        xkv_sb[(d, t)][:], start=(d == 0), stop=(d == NDT - 1))
                nc.vector.tensor_scalar_add(
                    kT[et][:, 512 * t:512 * (t + 1)], ps[:],
                    bk_sb[:, et:et + 1])

            def emit_C(tt):
                ps = pps.tile([128, 512], F32, tag="pp")
                t, q = tt // 4, tt % 4
                for d in range(NDT):
                    nc.tensor.matmul(
                        ps[:], xkv_sb[(d, t)][:, 128 * q:128 * (q + 1)],
                        wv_sb[d][:], start=(d == 0), stop=False)
                nc.tensor.matmul(ps[:], on_sb[:], bv_sb[:],
                                 start=False, stop=True)
                vv = v_sb[tt][:].rearrange("p (h w) -> p h w", w=VH)
                nc.vector.tensor_scalar_add(
                    vv[:, :, 0:HD],
                    ps[:].rearrange("p (h w) -> p h w", w=HD), 0.0)

            def emit_E(do, ic):
                ps = pps.tile([128, 512], F32, tag="pp")
                for e in range(NET):
                    nc.tensor.matmul(
                        ps[:], wo_sb[e][:, 128 * do:128 * (do + 1)],
                        aT[e][:, 512 * ic:512 * (ic + 1)],
                        start=(e == 0), stop=(e == NET - 1))
                o = os_pool.tile([128, 512], F32, tag="eo")
                nc.vector.tensor_scalar_add(o[:], ps[:], 0.0)
                nc.sync.dma_start(
                    outT[128 * do:128 * (do + 1), 512 * ic:512 * (ic + 1)],
                    o[:])

            def run_filler(f):
                kind = f[0]
                if kind == "A":
                    emit_A(f[1], f[2])
                elif kind == "B":
                    emit_B(f[1], f[2])
                elif kind == "C":
                    emit_C(f[1])
                else:
                    emit_E(f[1], f[2])

            # ---- prologue: qT[0], kT[0], v_sb[0..4] ----
            emit_A(0, 0)
            emit_B(0, 0)
            emit_C(0)
            emit_A(0, 1)
            emit_B(0, 1)
            emit_C(1)
            emit_C(2)
            emit_A(0, 2)
            emit_B(0, 2)
            emit_C(3)
            emit_C(4)
            emit_A(0, 3)
            emit_B(0, 3)

            # ---- phase D with interleaved fillers ----
            work = []   # (et, ic, jb, dlt, msk, first, last)
            for et in range(NET):
                for ic in range(NIC):
                    lst = groups[ic]
                    for (jb, dlt, msk) in lst:
                        work.append((et, ic, jb, dlt, msk,
                                     jb == lst[0][0], jb == lst[-1][0]))
            gpe = len(work) // NET            # groups per et

            fillers = {
                0: ([("C", tt) for tt in range(5, NJB)]
                    + [x for t in range(NIC) for x in (("A", 1, t), ("B", 1, t))]),
                1: [x for t in range(NIC) for x in (("A", 2, t), ("B", 2, t))],
                2: [x for t in range(NIC) for x in (("A", 3, t), ("B", 3, t))],
                3: [],
            }

            ot = {}           # (et, ic) -> (otA, otB)
            pending = {}      # n -> pt tile
            queue = []
            DEPTH = 5

            def emit_div(et, ic):
                otA, otB = ot.pop((et, ic))
                us, ds = [], []
                # two copies free the psum slot fast; the denominator row
                # goes to a base-partition-0 tile (reciprocal_approx_fast
                # mishandles nonzero base partitions)
                for o in (otA, otB):
                    u = dv_pool.tile([64, 512], F32, tag="u")
                    nc.vector.tensor_scalar_add(u[:], o[0:64, :], 0.0)
                    d = dv_pool.tile([1, 512], F32, tag="d")
                    nc.vector.tensor_scalar_add(d[:], o[64:65, :], 0.0)
                    us.append(u)
                    ds.append(d)
                rs = []
                for d in ds:
                    r = dv_pool.tile([1, 512], F32, tag="r")
                    nc.vector.reciprocal_approx_fast(r[:], d[:])
                    rs.append(r)
                rbs = []
                for r in rs:
                    rb = dv_pool.tile([64, 512], F32, tag="rb")
                    nc.gpsimd.partition_broadcast(rb[:], r[:])
                    rbs.append(rb)
                for half, (u, rb) in enumerate(zip(us, rbs)):
                    nc.vector.tensor_tensor(
                        aT[et][64 * half:64 * half + 64,
                               512 * ic:512 * (ic + 1)],
                        u[:], rb[:], MUL)
                if et == NET - 1:
                    queue.extend(("E", do, ic) for do in range(NDT))

            def emit_stage2(n):
                et, ic, jb, dlt, msk, first, last = work[n]
                pt = pending.pop(n)
                if first:
                    ot[(et, ic)] = (
                        ops.tile([65, 512], F32, tag="ot", name=f"oA{et}_{ic}"),
                        ops.tile([65, 512], F32, tag="ot", name=f"oB{et}_{ic}"))
                otA, otB = ot[(et, ic)]
                hA, hB = 2 * et, 2 * et + 1
                nc.tensor.matmul(
                    otA[:, dlt:512], v_sb[jb][:, VH * hA:VH * hA + VH],
                    pt[:, dlt:512], start=first, stop=last,
                    skip_group_check=True)
                nc.tensor.matmul(
                    otB[:, dlt:512], v_sb[jb][:, VH * hB:VH * hB + VH],
                    pt[:, 512 + dlt:1024], start=first, stop=last,
                    skip_group_check=True)
                if last:
                    emit_div(et, ic)

            acc = 0.0
            for n, (et, ic, jb, dlt, msk, first, last) in enumerate(work):
                g = n % gpe
                if g == 0:
                    queue.extend(fillers[et])
                # adaptive filler pacing: drain queue evenly over this et
                acc += len(queue) / max(1, gpe - g)
                while acc >= 1.0 and queue:
                    run_filler(queue.pop(0))
                    acc -= 1.0

                st = sps.tile([128, 1024], F32, tag="st")
                nc.tensor.matmul(
                    st[:, dlt:512],
                    kT[et][0:64, 128 * jb:128 * (jb + 1)],
                    qT[et][0:64, 512 * ic + dlt:512 * (ic + 1)],
                    start=True, stop=True)
                nc.tensor.matmul(
                    st[:, 512 + dlt:1024],
                    kT[et][64:128, 128 * jb:128 * (jb + 1)],
                    qT[et][64:128, 512 * ic + dlt:512 * (ic + 1)],
                    start=True, stop=True)
                pt = pt_pool.tile([128, 1024], BF16, tag="pt")
                nc.scalar.activation(pt[:, dlt:1024], st[:, dlt:1024],
                                     AF.Exp, scale=SCALE)
                if msk:
                    k = dpos[128 * jb - L - 512 * ic]
                    w = min(dlt + 128, 512) - dlt
                    for off in (0, 512):
                        nc.vector.tensor_tensor(
                            pt[:, off + dlt:off + dlt + w],
                            pt[:, off + dlt:off + dlt + w],
                            mk_sb[:, 512 * k + dlt:512 * k + dlt + w], MUL)
                pending[n] = pt
                if n >= DEPTH:
                    emit_stage2(n - DEPTH)
            for n in range(max(0, len(work) - DEPTH), len(work)):
                emit_stage2(n)
            while queue:
                run_filler(queue.pop(0))

    nc.compile()
    return nc, deltas


def _prep_core(query, key_value, Wq, bq, Wk, bk, Wv, bv, Wo, c, deltas, L):
    b, half = c // 2, c % 2
    hs = E_LOC * half
    f32, bf16 = np.float32, ml_dtypes.bfloat16
    xqT = np.ascontiguousarray(query[b].T).astype(bf16)
    xkvT = np.ascontiguousarray(key_value[b].T).astype(bf16)
    wqT = np.ascontiguousarray(Wq[hs:hs + E_LOC].T).astype(bf16)
    wkT = np.ascontiguousarray(Wk[hs:hs + E_LOC].T).astype(bf16)
    wvT = np.ascontiguousarray(Wv[hs:hs + E_LOC].T).astype(bf16)
    bv_row = bv[hs:hs + E_LOC].reshape(1, E_LOC).astype(bf16)
    woT = np.ascontiguousarray(Wo[:, hs:hs + E_LOC].T).astype(bf16)
    bq4 = np.ascontiguousarray(bq[hs:hs + E_LOC].reshape(NET, 128).T, dtype=f32)
    bk4 = np.ascontiguousarray(bk[hs:hs + E_LOC].reshape(NET, 128).T, dtype=f32)
    nmask = max(1, len(deltas))
    masks = np.zeros((128, nmask * 512), dtype=bf16)
    jr = np.arange(128)[:, None]
    ir = np.arange(512)[None, :]
    for k, d in enumerate(deltas):
        masks[:, 512 * k:512 * (k + 1)] = (jr <= ir - d).astype(bf16)
    return {"xqT": xqT, "xkvT": xkvT, "wqT": wqT, "wkT": wkT, "wvT": wvT,
            "woT": woT, "bq4": bq4, "bk4": bk4, "bv_row": bv_row,
            "masks": masks}


def kernel(query, key_value, Wq, bq, Wk, bk, Wv, bv, Wo, bo, lookahead,
           _trace=False):
    L = int(lookahead)
    if L not in _CACHE:
        _CACHE[L] = _build(L)
    nc, deltas = _CACHE[L]

    args = [np.asarray(a, dtype=np.float32) for a in
            (query, key_value, Wq, bq, Wk, bk, Wv, bv, Wo)]
    in_maps = [_prep_core(*args, c, deltas, L) for c in range(8)]
    res = run_bass_kernel_spmd(nc, in_maps, core_ids=list(range(8)),
                               trace=_trace)
    bo = np.asarray(bo, dtype=np.float32)
    out = np.empty((B, T, D), dtype=np.float32)
    for b in range(B):
        pT = res.results[2 * b]["outT"] + res.results[2 * b + 1]["outT"]
        out[b] = pT.T + bo[None, :]
    if _trace:
        kernel.last_exec_time_ns = res.exec_time_ns
    return out


# revision 15
# speedup vs baseline: 1.0083x; 1.0083x over previous
"""Banded (lookahead) cross-attention on 8 Trainium2 NeuronCores.

Reference computation (B=4, T=2048, D=1024, H=16, hd=64):
    Q = query @ Wq.T + bq ; K = key_value @ Wk.T + bk ; V = key_value @ Wv.T + bv
    scores = Q K^T / sqrt(hd), masked to j <= i + lookahead
    out = softmax(scores) V, concat heads, @ Wo.T + bo
Sharding: 8 cores = (batch b = c//2) x (head-half = c%2, 8 heads each).
Host sums the two outT partials per batch and adds bo.

v2 design (single fused pipeline, all engines busy):
  - Attention phase D is scalar-bound (exp on ACT @1.2GHz). All other
    scalar work (bias-adds, psum->sbuf copies) moved to DVE so ACT does
    exp only.
  - QK^T packs both heads of a pair concurrently on the PE: per-head
    contraction is hd=64, and bass auto-derives tile_position from the
    base partition (head A rows 0:64 -> row tiles 0-1, head B rows
    64:128 -> tiles 2-3), so back-to-back issue runs them in parallel.
  - Projection/output matmul bundles (A/B/C/E) are interleaved into D's
    emission as tensor-engine filler, keeping the PE busy (HAM warm)
    while ACT chews exp.
  - Everything bf16 (x, weights, P, V, aT, Wo); psum f32; div chain
    recip(DVE) -> partition_broadcast(GPSIMD) -> mult(DVE).
"""

import sys

for _p in ("/opt/trn_rl_repo", "/opt/pypackages"):
    if _p not in sys.path:
        sys.path.append(_p)

import numpy as np
import ml_dtypes

import concourse.bass as bass
import concourse.tile as tile
from concourse import bacc, mybir
from concourse.bass_utils import run_bass_kernel_spmd

F32 = mybir.dt.float32
BF16 = mybir.dt.bfloat16
AF = mybir.ActivationFunctionType
MUL = mybir.AluOpType.mult

B, T, D = 4, 2048, 1024
H, HD = 16, 64
H_LOC = 8                    # heads per core
E_LOC = H_LOC * HD           # 512 projected dims per core
NJB = T // 128               # 16 j-blocks
NIC = T // 512               # 4 i-chunks
NDT = D // 128               # 8 contraction tiles
NET = E_LOC // 128           # 4 e-tiles (head-pairs)
SCALE = HD ** -0.5
VW = H_LOC * (HD + 1)        # 520 v_sb layout width
VH = HD + 1                  # 65

_CACHE = {}


def _groups(L):
    """Per i-chunk: list of (jb, delta, masked); delta = first valid column
    offset inside the 512-wide chunk (0 for dense)."""
    out = []
    deltas = set()
    for ic in range(NIC):
        i0 = 512 * ic
        lst = []
        for jb in range(NJB):
            j0 = 128 * jb
            if i0 + 511 + L < j0:
                break                          # fully masked from here on
            if j0 + 127 <= i0 + L:
                lst.append((jb, 0, False))     # dense
            else:
                d = j0 - L - i0
                lst.append((jb, max(d, 0), True))
                deltas.add(d)
        out.append(lst)
    return out, sorted(deltas)


def _build(L):
    groups, deltas = _groups(L)
    dpos = {d: k for k, d in enumerate(deltas)}
    nmask = max(1, len(deltas))

    nc = bacc.Bacc("TRN2", target_bir_lowering=False, debug=False)
    xqT = nc.dram_tensor("xqT", [D, T], BF16, kind="ExternalInput").ap()
    xkvT = nc.dram_tensor("xkvT", [D, T], BF16, kind="ExternalInput").ap()
    wqT = nc.dram_tensor("wqT", [D, E_LOC], BF16, kind="ExternalInput").ap()
    wkT = nc.dram_tensor("wkT", [D, E_LOC], BF16, kind="ExternalInput").ap()
    wvT = nc.dram_tensor("wvT", [D, E_LOC], BF16, kind="ExternalInput").ap()
    woT = nc.dram_tensor("woT", [E_LOC, D], BF16, kind="ExternalInput").ap()
    bq4 = nc.dram_tensor("bq4", [128, NET], F32, kind="ExternalInput").ap()
    bk4 = nc.dram_tensor("bk4", [128, NET], F32, kind="ExternalInput").ap()
    bv_row = nc.dram_tensor("bv_row", [1, E_LOC], BF16, kind="ExternalInput").ap()
    masks = nc.dram_tensor("masks", [128, nmask * 512], BF16,
                           kind="ExternalInput").ap()
    outT = nc.dram_tensor("outT", [D, T], F32, kind="ExternalOutput").ap()

    with tile.TileContext(nc) as tc:
        with tc.tile_pool(name="small", bufs=1) as small, \
             tc.tile_pool(name="persist", bufs=1) as persist, \
             tc.tile_pool(name="slabs", bufs=1) as slabs, \
             tc.tile_pool(name="ptp", bufs=7) as pt_pool, \
             tc.tile_pool(name="dv", bufs=2) as dv_pool, \
             tc.tile_pool(name="os", bufs=4) as os_pool, \
             tc.tile_pool(name="pps", bufs=2, space="PSUM") as pps, \
             tc.tile_pool(name="sps", bufs=2, space="PSUM") as sps, \
             tc.tile_pool(name="ops", bufs=2, space="PSUM") as ops:

            # ---- DMA issue order == first-need order ----
            wq_sb = [slabs.tile([128, E_LOC], BF16, tag=f"wq{d}", name=f"wq{d}")
                     for d in range(NDT)]
            xq_sb = {}
            xkv_sb = {}
            for t in range(NIC):
                for d in range(NDT):
                    xq_sb[(d, t)] = slabs.tile(
                        [128, 512], BF16, tag=f"xq{d}_{t}", name=f"xq{d}_{t}")
                    xkv_sb[(d, t)] = slabs.tile(
                        [128, 512], BF16, tag=f"xkv{d}_{t}", name=f"xkv{d}_{t}")
            wk_sb = [slabs.tile([128, E_LOC], BF16, tag=f"wk{d}", name=f"wk{d}")
                     for d in range(NDT)]
            wv_sb = [slabs.tile([128, E_LOC], BF16, tag=f"wv{d}", name=f"wv{d}")
                     for d in range(NDT)]
            wo_sb = [slabs.tile([128, D], BF16, tag=f"wo{e}", name=f"wo{e}")
                     for e in range(NET)]
            # interleave so A(0,0)'s first matmuls start after ~256KB
            for d in range(NDT):
                nc.sync.dma_start(wq_sb[d][:], wqT[128 * d:128 * (d + 1), :])
                nc.sync.dma_start(
                    xq_sb[(d, 0)][:], xqT[128 * d:128 * (d + 1), 0:512])
            for d in range(NDT):
                nc.sync.dma_start(wk_sb[d][:], wkT[128 * d:128 * (d + 1), :])
                nc.sync.dma_start(
                    xkv_sb[(d, 0)][:], xkvT[128 * d:128 * (d + 1), 0:512])
            for d in range(NDT):
                nc.sync.dma_start(wv_sb[d][:], wvT[128 * d:128 * (d + 1), :])
            bq_sb = small.tile([128, NET], F32, tag="bq")
            bk_sb = small.tile([128, NET], F32, tag="bk")
            bv_sb = small.tile([1, E_LOC], BF16, tag="bv")
            on_sb = small.tile([1, 128], BF16, tag="on")
            mk_sb = persist.tile([128, nmask * 512], BF16, tag="mk")
            nc.sync.dma_start(mk_sb[:], masks[:])
            nc.sync.dma_start(bq_sb[:], bq4[:])
            nc.sync.dma_start(bk_sb[:], bk4[:])
            nc.sync.dma_start(bv_sb[:], bv_row[:])
            for t in range(1, NIC):
                for d in range(NDT):
                    nc.sync.dma_start(
                        xq_sb[(d, t)][:],
                        xqT[128 * d:128 * (d + 1), 512 * t:512 * (t + 1)])
                for d in range(NDT):
                    nc.sync.dma_start(
                        xkv_sb[(d, t)][:],
                        xkvT[128 * d:128 * (d + 1), 512 * t:512 * (t + 1)])
            for e in range(NET):
                nc.sync.dma_start(wo_sb[e][:], woT[128 * e:128 * (e + 1), :])

            qT = [persist.tile([128, T], BF16, tag=f"qt{i}", name=f"qt{i}")
                  for i in range(NET)]
            kT = [persist.tile([128, T], BF16, tag=f"kt{i}", name=f"kt{i}")
                  for i in range(NET)]
            v_sb = [persist.tile([128, VW], BF16, tag=f"v{i}", name=f"v{i}")
                    for i in range(NJB)]
            aT = [persist.tile([128, T], BF16, tag=f"at{i}", name=f"at{i}")
                  for i in range(NET)]

            # ones: lhsT row for C's bias matmul; v ones columns (softmax
            # denominator accumulators) written once
            nc.gpsimd.memset(on_sb[:], 1.0)
            for tt in range(NJB):
                vv = v_sb[tt][:].rearrange("p (h w) -> p h w", w=VH)
                nc.gpsimd.memset(vv[:, :, HD:VH], 1.0)

            # ---- filler bundle emitters (projections / output) ----
            def emit_A(et, t):
                ps = pps.tile([128, 512], F32, tag="pp")
                for d in range(NDT):
                    nc.tensor.matmul(
                        ps[:], wq_sb[d][:, 128 * et:128 * (et + 1)],
                        xq_sb[(d, t)][:], start=(d == 0), stop=(d == NDT - 1))
                nc.vector.tensor_scalar_add(
                    qT[et][:, 512 * t:512 * (t + 1)], ps[:],
                    bq_sb[:, et:et + 1])

            def emit_B(et, t):
                ps = pps.tile([128, 512], F32, tag="pp")
                for d in range(NDT):
                    nc.tensor.matmul(
                        ps[:], wk_sb[d][:, 128 * et:128 * (et + 1)],
                        xkv_sb[(d, t)][:], start=(d == 0), stop=(d == NDT - 1))
                nc.vector.tensor_scalar_add(
                    kT[et][:, 512 * t:512 * (t + 1)], ps[:],
                    bk_sb[:, et:et + 1])

            def emit_C(tt):
                ps = pps.tile([128, 512], F32, tag="pp")
                t, q = tt // 4, tt % 4
                for d in range(NDT):
                    nc.tensor.matmul(
                        ps[:], xkv_sb[(d, t)][:, 128 * q:128 * (q + 1)],
                        wv_sb[d][:], start=(d == 0), stop=False)
                nc.tensor.matmul(ps[:], on_sb[:], bv_sb[:],
                                 start=False, stop=True)
                vv = v_sb[tt][:].rearrange("p (h w) -> p h w", w=VH)
                nc.vector.tensor_scalar_add(
                    vv[:, :, 0:HD],
                    ps[:].rearrange("p (h w) -> p h w", w=HD), 0.0)

            def emit_E(do, ic):
                ps = pps.tile([128, 512], F32, tag="pp")
                for e in range(NET):
                    nc.tensor.matmul(
                        ps[:], wo_sb[e][:, 128 * do:128 * (do + 1)],
                        aT[e][:, 512 * ic:512 * (ic + 1)],
                        start=(e == 0), stop=(e == NET - 1))
                o = os_pool.tile([128, 512], F32, tag="eo")
                nc.vector.tensor_scalar_add(o[:], ps[:], 0.0)
                nc.sync.dma_start(
                    outT[128 * do:128 * (do + 1), 512 * ic:512 * (ic + 1)],
                    o[:])

            def run_filler(f):
                kind = f[0]
                if kind == "A":
                    emit_A(f[1], f[2])
                elif kind == "B":
                    emit_B(f[1], f[2])
                elif kind == "C":
                    emit_C(f[1])
                else:
                    emit_E(f[1], f[2])

            # ---- prologue: qT[0], kT[0], v_sb[0..4] ----
            emit_A(0, 0)
            emit_B(0, 0)
            emit_C(0)
            emit_A(0, 1)
            emit_B(0, 1)
            emit_C(1)
            emit_C(2)
            emit_A(0, 2)
            emit_B(0, 2)
            emit_C(3)
            emit_C(4)
            emit_A(0, 3)
            emit_B(0, 3)

            # ---- phase D with interleaved fillers ----
            work = []   # (et, ic, jb, dlt, msk, first, last)
            for et in range(NET):
                for ic in range(NIC):
                    lst = groups[ic]
                    for (jb, dlt, msk) in lst:
                        work.append((et, ic, jb, dlt, msk,
                                     jb == lst[0][0], jb == lst[-1][0]))
            gpe = len(work) // NET            # groups per et

            fillers = {
                0: ([("C", tt) for tt in range(5, NJB)]
                    + [x for t in range(NIC) for x in (("A", 1, t), ("B", 1, t))]),
                1: [x for t in range(NIC) for x in (("A", 2, t), ("B", 2, t))],
                2: [x for t in range(NIC) for x in (("A", 3, t), ("B", 3, t))],
                3: [],
            }

            ot = {}           # (et, ic) -> (otA, otB)
            pending = {}      # n -> pt tile
            queue = []
            DEPTH = 4

            def emit_div(et, ic):
                otA, otB = ot.pop((et, ic))
                us, ds = [], []
                # two copies free the psum slot fast; the denominator row
                # goes to a base-partition-0 tile (reciprocal_approx_fast
                # mishandles nonzero base partitions)
                for o in (otA, otB):
                    u = dv_pool.tile([64, 512], F32, tag="u")
                    nc.vector.tensor_scalar_add(u[:], o[0:64, :], 0.0)
                    d = dv_pool.tile([1, 512], F32, tag="d")
                    nc.vector.tensor_scalar_add(d[:], o[64:65, :], 0.0)
                    us.append(u)
                    ds.append(d)
                rs = []
                for d in ds:
                    r = dv_pool.tile([1, 512], F32, tag="r")
                    nc.vector.reciprocal_approx_fast(r[:], d[:])
                    rs.append(r)
                rbs = []
                for r in rs:
                    rb = dv_pool.tile([64, 512], F32, tag="rb")
                    nc.gpsimd.partition_broadcast(rb[:], r[:])
                    rbs.append(rb)
                for half, (u, rb) in enumerate(zip(us, rbs)):
                    nc.vector.tensor_tensor(
                        aT[et][64 * half:64 * half + 64,
                               512 * ic:512 * (ic + 1)],
                        u[:], rb[:], MUL)
                if et == NET - 1:
                    queue.extend(("E", do, ic) for do in range(NDT))

            def emit_stage2(n):
                et, ic, jb, dlt, msk, first, last = work[n]
                pt = pending.pop(n)
                if first:
                    ot[(et, ic)] = (
                        ops.tile([65, 512], F32, tag="ot", name=f"oA{et}_{ic}"),
                        ops.tile([65, 512], F32, tag="ot", name=f"oB{et}_{ic}"))
                otA, otB = ot[(et, ic)]
                hA, hB = 2 * et, 2 * et + 1
                nc.tensor.matmul(
                    otA[:, dlt:512], v_sb[jb][:, VH * hA:VH * hA + VH],
                    pt[:, dlt:512], start=first, stop=last,
                    skip_group_check=True)
                nc.tensor.matmul(
                    otB[:, dlt:512], v_sb[jb][:, VH * hB:VH * hB + VH],
                    pt[:, 512 + dlt:1024], start=first, stop=last,
                    skip_group_check=True)
                if last:
                    emit_div(et, ic)

            acc = 0.0
            for n, (et, ic, jb, dlt, msk, first, last) in enumerate(work):
                g = n % gpe
                if g == 0:
                    queue.extend(fillers[et])
                # adaptive filler pacing: drain queue evenly over this et
                acc += len(queue) / max(1, gpe - g)
                while acc >= 1.0 and queue:
                    run_filler(queue.pop(0))
                    acc -= 1.0

                st = sps.tile([128, 1024], F32, tag="st")
                nc.tensor.matmul(
                    st[:, dlt:512],
                    kT[et][0:64, 128 * jb:128 * (jb + 1)],
                    qT[et][0:64, 512 * ic + dlt:512 * (ic + 1)],
                    start=True, stop=True)
                nc.tensor.matmul(
                    st[:, 512 + dlt:1024],
                    kT[et][64:128, 128 * jb:128 * (jb + 1)],
                    qT[et][64:128, 512 * ic + dlt:512 * (ic + 1)],
                    start=True, stop=True)
                pt = pt_pool.tile([128, 1024], BF16, tag="pt")
                nc.scalar.activation(pt[:, dlt:1024], st[:, dlt:1024],
                                     AF.Exp, scale=SCALE)
                if msk:
                    k = dpos[128 * jb - L - 512 * ic]
                    w = min(dlt + 128, 512) - dlt
                    for off in (0, 512):
                        nc.vector.tensor_tensor(
                            pt[:, off + dlt:off + dlt + w],
                            pt[:, off + dlt:off + dlt + w],
                            mk_sb[:, 512 * k + dlt:512 * k + dlt + w], MUL)
                pending[n] = pt
                if n >= DEPTH:
                    emit_stage2(n - DEPTH)
            for n in range(max(0, len(work) - DEPTH), len(work)):
                emit_stage2(n)
            while queue:
                run_filler(queue.pop(0))

    nc.compile()
    return nc, deltas


def _prep_core(query, key_value, Wq, bq, Wk, bk, Wv, bv, Wo, c, deltas, L):
    b, half = c // 2, c % 2
    hs = E_LOC * half
    f32, bf16 = np.float32, ml_dtypes.bfloat16
    xqT = np.ascontiguousarray(query[b].T).astype(bf16)
    xkvT = np.ascontiguousarray(key_value[b].T).astype(bf16)
    wqT = np.ascontiguousarray(Wq[hs:hs + E_LOC].T).astype(bf16)
    wkT = np.ascontiguousarray(Wk[hs:hs + E_LOC].T).astype(bf16)
    wvT = np.ascontiguousarray(Wv[hs:hs + E_LOC].T).astype(bf16)
    bv_row = bv[hs:hs + E_LOC].reshape(1, E_LOC).astype(bf16)
    woT = np.ascontiguousarray(Wo[:, hs:hs + E_LOC].T).astype(bf16)
    bq4 = np.ascontiguousarray(bq[hs:hs + E_LOC].reshape(NET, 128).T, dtype=f32)
    bk4 = np.ascontiguousarray(bk[hs:hs + E_LOC].reshape(NET, 128).T, dtype=f32)
    nmask = max(1, len(deltas))
    masks = np.zeros((128, nmask * 512), dtype=bf16)
    jr = np.arange(128)[:, None]
    ir = np.arange(512)[None, :]
    for k, d in enumerate(deltas):
        masks[:, 512 * k:512 * (k + 1)] = (jr <= ir - d).astype(bf16)
    return {"xqT": xqT, "xkvT": xkvT, "wqT": wqT, "wkT": wkT, "wvT": wvT,
            "woT": woT, "bq4": bq4, "bk4": bk4, "bv_row": bv_row,
            "masks": masks}


def kernel(query, key_value, Wq, bq, Wk, bk, Wv, bv, Wo, bo, lookahead,
           _trace=False):
    L = int(lookahead)
    if L not in _CACHE:
        _CACHE[L] = _build(L)
    nc, deltas = _CACHE[L]

    args = [np.asarray(a, dtype=np.float32) for a in
            (query, key_value, Wq, bq, Wk, bk, Wv, bv, Wo)]
    in_maps = [_prep_core(*args, c, deltas, L) for c in range(8)]
    res = run_bass_kernel_spmd(nc, in_maps, core_ids=list(range(8)),
                               trace=_trace)
    bo = np.asarray(bo, dtype=np.float32)
    out = np.empty((B, T, D), dtype=np.float32)
    for b in range(B):
        pT = res.results[2 * b]["outT"] + res.results[2 * b + 1]["outT"]
        out[b] = pT.T + bo[None, :]
    if _trace:
        kernel.last_exec_time_ns = res.exec_time_ns
    return out
